# revision 1
# baseline (speedup 1.0000x reference)
"""ContextualRoIAlign Trainium2 kernel.

Problem (hardcoded): B=2, C=256, H=W=56, N=64 boxes, M=8 gt boxes, P=7.
out[b,n,c,p,q] = roi_align(fm[b], box_n)[c,p,q]
                 + mean_m roi_align(fm[b], union(box_n, gt_m))[c,p,q]

Decomposition: roi_align separates per axis into small interpolation
matrices Ay, Ax ([R,7,56], host-precomputed exactly like the reference):
  out[r,c,p,q] = sum_h sum_w Ay[r,p,h] * fm[c,h,w] * Ax[r,q,w]
The 1/M mean weight is folded into Ax of the context rois, and the 9-roi
group sum is accumulated in PSUM.

Sharding: 8 cores; core k handles image k//4, box groups [16*(k%4), +16)
=> 144 rois per core (16 groups x (1 box + 8 ctx)). fm replicated per
image (4 cores each).

Device program per core (all fp32 by default):
  Stage 1 (contract h): weights = fm channel-pair [h=56, 128] where col
    c_loc*64+w holds fm[2i+c_loc, h, w]; rhs = AyT [56, 504] (all rois'
    (r,p) columns, 2 chunks); psum [128, 504] -> TMP[128, i_loc, 1008].
    TMP partition psi*64+w holds tmp[c=2i+psi, w, r, p]: w ends up on
    partitions with no inter-stage transpose.
  Stage 2 (contract w): weights = TMP[psi*64:+56, :, r*7:+7] ([56,16,7],
    M=112=(c_i,p)); rhs = AxT[psi*64:+56, r*7:+7] ([56,7]); 9 rois of a
    group accumulate into one psum [112,7] = final out chunk.
"""
import os
import numpy as np

P = 7
B, C, H, W, N, M = 2, 256, 56, 56, 64, 8
NCORES = 8
GROUPS_PER_CORE = 16
ROIS_PER_GROUP = 9
R_CORE = GROUPS_PER_CORE * ROIS_PER_GROUP   # 144
RP = R_CORE * P                              # 1008
NPAIRS = 128
WIN = 16                                     # channel pairs per window
NWIN = NPAIRS // WIN                         # 8
NCHUNK = 504                                 # stage-1 rhs cols per matmul


# ---------------------------------------------------------------- host prep

def _axis_weights(start, length, dim):
    """Exact numpy port of the reference's _axis_weights (float32)."""
    start = start.astype(np.float32)
    length = length.astype(np.float32)
    R = start.shape[0]
    S = int(np.ceil(dim / P))
    bin_sz = length / np.float32(P)
    grid = np.ceil(length / np.float32(P)).astype(np.int32)
    g = grid.astype(np.float32)[:, None, None]
    s = np.arange(S, dtype=np.float32)
    ph = np.arange(P, dtype=np.float32)
    coord = (start[:, None, None] + ph[None, :, None] * bin_sz[:, None, None]
             + (s[None, None, :] + np.float32(0.5)) * bin_sz[:, None, None] / g)
    valid = (coord >= -1.0) & (coord <= dim)
    c = np.maximum(coord, np.float32(0.0))
    low = np.floor(c).astype(np.int32)
    hi_clamp = low >= dim - 1
    low = np.where(hi_clamp, dim - 1, low)
    high = np.where(hi_clamp, dim - 1, low + 1)
    cv = np.where(hi_clamp, low.astype(np.float32), c)
    l = cv - low.astype(np.float32)
    smask = (s[None, None, :] < g) & valid
    w = smask.astype(np.float32) / g
    w_low = ((np.float32(1.0) - l) * w).astype(np.float32)
    w_high = (l * w).astype(np.float32)
    A = np.zeros((R, P, dim), dtype=np.float32)
    r_idx = np.broadcast_to(np.arange(R)[:, None, None], low.shape)
    p_idx = np.broadcast_to(np.arange(P)[None, :, None], low.shape)
    np.add.at(A, (r_idx, p_idx, low), w_low)
    np.add.at(A, (r_idx, p_idx, high), w_high)
    return A


def _prep_core(fm_b, boxes_b, gt_b, g0):
    b = boxes_b.astype(np.float32)
    g = gt_b.astype(np.float32)
    x1 = np.minimum(b[:, None, 0], g[None, :, 0])
    y1 = np.minimum(b[:, None, 1], g[None, :, 1])
    x2 = np.maximum(b[:, None, 2], g[None, :, 2])
    y2 = np.maximum(b[:, None, 3], g[None, :, 3])
    ctx = np.stack([x1, y1, x2, y2], axis=-1)                 # [N,M,4]
    rois = np.concatenate([b[:, None, :], ctx], axis=1)       # [N,9,4]
    wts = np.full((N, ROIS_PER_GROUP), np.float32(1.0 / M), dtype=np.float32)
    wts[:, 0] = np.float32(1.0)

    rois = rois[g0:g0 + GROUPS_PER_CORE].reshape(R_CORE, 4)
    wts = wts[g0:g0 + GROUPS_PER_CORE].reshape(R_CORE)
    x1, y1, x2, y2 = rois[:, 0], rois[:, 1], rois[:, 2], rois[:, 3]
    roi_w = np.maximum(x2 - x1, np.float32(1.0))
    roi_h = np.maximum(y2 - y1, np.float32(1.0))
    Ay = _axis_weights(y1, roi_h, H)
    Ax = _axis_weights(x1, roi_w, W) * wts[:, None, None]

    AyT = np.ascontiguousarray(Ay.transpose(2, 0, 1).reshape(H, RP))
    # Ax with q padded 7->8 (fp32r matmuls need even free sizes)
    Ax8 = np.zeros((R_CORE, 8, W), dtype=np.float32)
    Ax8[:, :P] = Ax
    AxT = Ax8.transpose(2, 0, 1).reshape(W, R_CORE * 8)
    AxT_dup = np.zeros((128, R_CORE * 8), dtype=np.float32)
    AxT_dup[0:56] = AxT
    AxT_dup[64:120] = AxT

    F3 = np.zeros((H, NPAIRS, 128), dtype=np.float32)
    fmT = fm_b.transpose(1, 0, 2)                              # [h, c, w]
    F3[:, :, 0:56] = fmT[:, 0::2, :]
    F3[:, :, 64:120] = fmT[:, 1::2, :]
    return np.ascontiguousarray(F3), AyT, np.ascontiguousarray(AxT_dup)


def _unpack_core_out(OUT):
    """OUT [112,16,112] -> [16, 256, 7, 7]."""
    a = OUT.reshape(WIN, P, GROUPS_PER_CORE, 2, NWIN, P)
    a = a.transpose(2, 4, 0, 3, 1, 5)            # [g, win, c_i, psi, p, q]
    return np.ascontiguousarray(a.reshape(GROUPS_PER_CORE, C, P, P))


# ---------------------------------------------------------------- program

_PROGRAM = None


def _build_program():
    import concourse.bacc as bacc
    import concourse.tile as tile
    import concourse.mybir as mybir

    f32 = mybir.dt.float32
    dts = {"float32": mybir.dt.float32, "float32r": mybir.dt.float32r,
           "bfloat16": mybir.dt.bfloat16}
    s1_dt = dts[os.environ.get("ROI_S1_DTYPE", "float32r")]
    s2_dt = dts[os.environ.get("ROI_S2_DTYPE", "float32r")]

    nc = bacc.Bacc("TRN2", target_bir_lowering=False, debug=False,
                   enable_asserts=False)
    f3_d = nc.dram_tensor("f3", [H, NPAIRS, 128], f32, kind="ExternalInput").ap()
    ayt_d = nc.dram_tensor("ayt", [H, RP], f32, kind="ExternalInput").ap()
    axt_d = nc.dram_tensor("axt", [128, R_CORE * 8], f32, kind="ExternalInput").ap()
    out_d = nc.dram_tensor("out", [112, GROUPS_PER_CORE, 112], f32,
                           kind="ExternalOutput").ap()

    with tile.TileContext(nc) as tc:
        with tc.tile_pool(name="const", bufs=1) as cpool, \
             tc.tile_pool(name="fmw", bufs=2) as fpool, \
             tc.tile_pool(name="tmp", bufs=2) as tpool, \
             tc.tile_pool(name="outp", bufs=1) as opool, \
             tc.tile_pool(name="ps1", bufs=3, space="PSUM") as ps1p, \
             tc.tile_pool(name="ps2", bufs=4, space="PSUM") as ps2p:

            AyT_raw = cpool.tile([H, RP], f32)
            nc.sync.dma_start(AyT_raw[:], ayt_d)
            AxT_raw = cpool.tile([128, R_CORE * 8], f32)
            nc.sync.dma_start(AxT_raw[:], axt_d)
            if s1_dt != f32:
                AyT = cpool.tile([H, RP], s1_dt)
                nc.vector.tensor_copy(out=AyT[:], in_=AyT_raw[:])
            else:
                AyT = AyT_raw
            if s2_dt != f32:
                AxT = cpool.tile([128, R_CORE * 8], s2_dt)
                nc.vector.tensor_copy(out=AxT[:], in_=AxT_raw[:])
            else:
                AxT = AxT_raw
            OUT = opool.tile([112, GROUPS_PER_CORE, 112], f32)

            ncopy = 0
            for win in range(NWIN):
                F3raw = fpool.tile([H, WIN, 128], f32, tag="f3raw")
                nc.sync.dma_start(F3raw[:], f3_d[:, win * WIN:(win + 1) * WIN, :])
                if s1_dt != f32:
                    F3w = fpool.tile([H, WIN, 128], s1_dt, tag="f3w")
                    nc.scalar.copy(out=F3w[:], in_=F3raw[:])
                else:
                    F3w = F3raw
                # TMP[psi*64+w, r, c_i*7+p] = tmp[c=2*(win*16+c_i)+psi, w, r, p]
                # (layout r-major so a stage-2 weights slice is one
                # contiguous 112-element free dim)
                TMP = tpool.tile([128, R_CORE, WIN * P], s2_dt, tag="tmp")
                for il in range(WIN):
                    for ch in range(2):
                        ps = ps1p.tile([128, NCHUNK], f32, tag="ps1")
                        nc.tensor.matmul(
                            ps[:],
                            F3w[:, il, :],
                            AyT[:, ch * NCHUNK:(ch + 1) * NCHUNK],
                            start=True, stop=True)
                        dst = TMP[:, ch * 72:(ch + 1) * 72, il * P:(il + 1) * P]
                        if ncopy % 2 == 0:
                            nc.vector.tensor_copy(out=dst, in_=ps[:])
                        else:
                            nc.scalar.copy(out=dst, in_=ps[:])
                        ncopy += 1
                for g in range(GROUPS_PER_CORE):
                    for psi in range(2):
                        ps2 = ps2p.tile([112, 8], f32, tag="ps2")
                        for j in range(ROIS_PER_GROUP):
                            r = g * ROIS_PER_GROUP + j
                            lhsT = TMP[psi * 64:psi * 64 + 56, r, :]
                            rhs = AxT[psi * 64:psi * 64 + 56, r * 8:(r + 1) * 8]
                            nc.tensor.matmul(
                                ps2[:], lhsT, rhs,
                                start=(j == 0), stop=(j == ROIS_PER_GROUP - 1))
                        nc.any.tensor_copy(
                            out=OUT[:, g, (psi * NWIN + win) * P:(psi * NWIN + win + 1) * P],
                            in_=ps2[:, 0:P])
            nc.sync.dma_start(out_d, OUT[:])

    nc.compile()
    return nc


LAST_RESULT = None


def _ensure_axon_hooks_shim():
    """concourse's axon trace path imports antenv.axon_hooks, which this
    image's antenv package lacks; provide a minimal registry so a stray
    BASS_TRACE=1 in the environment cannot crash the kernel."""
    try:
        import antenv  # noqa: F401
        import antenv.axon_hooks  # noqa: F401
        return
    except ImportError:
        pass
    try:
        import sys
        import types
        import antenv
        mod = types.ModuleType("antenv.axon_hooks")
        mod._hook = None
        mod.get_axon_ntff_profile_hook = lambda: mod._hook

        def _set(h):
            mod._hook = h

        mod.set_axon_ntff_profile_hook = _set
        sys.modules["antenv.axon_hooks"] = mod
        antenv.axon_hooks = mod
    except Exception:
        pass


def kernel(feature_map, boxes, gt_boxes):
    global _PROGRAM, LAST_RESULT
    _ensure_axon_hooks_shim()
    feature_map = np.asarray(feature_map, dtype=np.float32)
    boxes = np.asarray(boxes, dtype=np.float32)
    gt_boxes = np.asarray(gt_boxes, dtype=np.float32)

    from concourse.bass_utils import run_bass_kernel_spmd

    if _PROGRAM is None:
        _PROGRAM = _build_program()
    nc = _PROGRAM

    in_maps = []
    for k in range(NCORES):
        b = k // 4
        g0 = (k % 4) * GROUPS_PER_CORE
        F3, AyT, AxT_dup = _prep_core(feature_map[b], boxes[b], gt_boxes[b], g0)
        in_maps.append({"f3": F3, "ayt": AyT, "axt": AxT_dup})

    trace = bool(int(os.environ.get("ROI_TRACE", "0")))
    res = run_bass_kernel_spmd(nc, in_maps, list(range(NCORES)), trace=trace)
    LAST_RESULT = res

    out = np.zeros((B, N, C, P, P), dtype=np.float32)
    for k in range(NCORES):
        b = k // 4
        g0 = (k % 4) * GROUPS_PER_CORE
        out[b, g0:g0 + GROUPS_PER_CORE] = _unpack_core_out(res.results[k]["out"])
    return out



# revision 4
# speedup vs baseline: 8.7041x; 8.7041x over previous
"""ContextualRoIAlign Trainium2 kernel — fused group-kernel formulation.

Problem (hardcoded): B=2, C=256, H=W=56, N=64 boxes, M=8 gt boxes, P=7.
out[b,n,c,p,q] = roi_align(fm[b], box_n)[c,p,q]
                 + mean_m roi_align(fm[b], union(box_n, gt_m))[c,p,q]

roi_align separates per axis into interpolation matrices Ay, Ax
([7,dim], host-precomputed exactly like the reference), so each roi is
out_r = Ay_r @ fm @ Ax_r^T.  The whole 9-roi group sum (box + mean of
its 8 ctx unions, 1/M folded into Ax) collapses into ONE dense spatial
kernel per group:

    G_g[(h,w),(p,q)] = sum_j Ay_j[p,h] * Ax_j[q,w]          (host, ~44 MFLOP/core)
    out_g[c,(p,q)]   = sum_hw fm[c,(h,w)] * G_g[(h,w),(p,q)] (device)

The device then does a single [256 x 3136] @ [3136 x 784] matmul per
core at full 128x128 PE utilization: hw is chunked into 25 K-tiles of
128 accumulated in PSUM; fm chunk is the stationary operand (shared by
all 16 groups); G streams as the moving operand (784 = 16 groups x 49
output pixels, split in two 392-column halves to fit a PSUM bank).
~100 matmuls total instead of the ~2800 tiny per-roi matmuls of the
two-stage formulation.

Sharding: 8 cores; core k handles image k//4, box groups [16*(k%4), +16).

DMA: G ([3136, 784]) is the dominant stream; it is shipped in 5 large
super-chunks (5 K-tiles each) to stay near peak HBM bandwidth while
overlapping with PE compute. Default io dtype bf16 (G/fm magnitudes are
O(1); psum accumulates fp32); ROI_DTYPE=float32r keeps full precision
at 2x the DMA cost.
"""
import os
import numpy as np

P = 7
B, C, H, W, N, M = 2, 256, 56, 56, 64, 8
NCORES = 8
GROUPS_PER_CORE = 16
ROIS_PER_GROUP = 9          # 1 box + 8 ctx unions
R_CORE = GROUPS_PER_CORE * ROIS_PER_GROUP   # 144
HW = H * W                  # 3136
KC = 128                    # contraction tile (partition dim)
NCHUNK = 25                 # ceil(3136/128)
HWPAD = NCHUNK * KC         # 3200
SUPER = 5                   # K-tiles per DMA super-chunk
NSUPER = NCHUNK // SUPER    # 5
PQ = P * P                  # 49
COLS = GROUPS_PER_CORE * PQ             # 784 moving columns
COLH = COLS // 2                        # 392 (<=512 psum bank)


# ---------------------------------------------------------------- host prep

def _axis_weights(start, length, dim):
    """Exact numpy port of the reference's _axis_weights (float32)."""
    start = start.astype(np.float32)
    length = length.astype(np.float32)
    R = start.shape[0]
    S = int(np.ceil(dim / P))
    bin_sz = length / np.float32(P)
    grid = np.ceil(length / np.float32(P)).astype(np.int32)
    g = grid.astype(np.float32)[:, None, None]
    s = np.arange(S, dtype=np.float32)
    ph = np.arange(P, dtype=np.float32)
    coord = (start[:, None, None] + ph[None, :, None] * bin_sz[:, None, None]
             + (s[None, None, :] + np.float32(0.5)) * bin_sz[:, None, None] / g)
    valid = (coord >= -1.0) & (coord <= dim)
    c = np.maximum(coord, np.float32(0.0))
    low = np.floor(c).astype(np.int32)
    hi_clamp = low >= dim - 1
    low = np.where(hi_clamp, dim - 1, low)
    high = np.where(hi_clamp, dim - 1, low + 1)
    cv = np.where(hi_clamp, low.astype(np.float32), c)
    l = cv - low.astype(np.float32)
    smask = (s[None, None, :] < g) & valid
    w = smask.astype(np.float32) / g
    w_low = ((np.float32(1.0) - l) * w).astype(np.float32)
    w_high = (l * w).astype(np.float32)
    A = np.zeros((R, P, dim), dtype=np.float32)
    r_idx = np.broadcast_to(np.arange(R)[:, None, None], low.shape)
    p_idx = np.broadcast_to(np.arange(P)[None, :, None], low.shape)
    np.add.at(A, (r_idx, p_idx, low), w_low)
    np.add.at(A, (r_idx, p_idx, high), w_high)
    return A


def _prep_core(fm_b, boxes_b, gt_b, g0, np_dt):
    """Per-core inputs: FM [128, 25, 256], G [128, 25, 784]."""
    b = boxes_b.astype(np.float32)
    g = gt_b.astype(np.float32)
    x1 = np.minimum(b[:, None, 0], g[None, :, 0])
    y1 = np.minimum(b[:, None, 1], g[None, :, 1])
    x2 = np.maximum(b[:, None, 2], g[None, :, 2])
    y2 = np.maximum(b[:, None, 3], g[None, :, 3])
    ctx = np.stack([x1, y1, x2, y2], axis=-1)                 # [N,M,4]
    rois = np.concatenate([b[:, None, :], ctx], axis=1)       # [N,9,4]
    wts = np.full((N, ROIS_PER_GROUP), np.float32(1.0 / M), dtype=np.float32)
    wts[:, 0] = np.float32(1.0)

    rois = rois[g0:g0 + GROUPS_PER_CORE].reshape(R_CORE, 4)
    wts = wts[g0:g0 + GROUPS_PER_CORE].reshape(R_CORE)
    x1, y1, x2, y2 = rois[:, 0], rois[:, 1], rois[:, 2], rois[:, 3]
    roi_w = np.maximum(x2 - x1, np.float32(1.0))
    roi_h = np.maximum(y2 - y1, np.float32(1.0))
    Ay = _axis_weights(y1, roi_h, H)                          # [144,7,56]
    Ax = _axis_weights(x1, roi_w, W) * wts[:, None, None]     # [144,7,56]

    # G_g[p,h,q,w] = sum_j Ay[j,p,h] Ax[j,q,w]  (rank-9 per group)
    Ayg = Ay.reshape(GROUPS_PER_CORE, ROIS_PER_GROUP, P * H)
    Axg = Ax.reshape(GROUPS_PER_CORE, ROIS_PER_GROUP, P * W)
    G2 = np.matmul(Ayg.transpose(0, 2, 1), Axg)               # [16, 392, 392]
    G5 = G2.reshape(GROUPS_PER_CORE, P, H, P, W)
    # -> [(h,w), (g,p,q)]
    Ghw = np.ascontiguousarray(G5.transpose(2, 4, 0, 1, 3)).reshape(HW, COLS)
    Gp = np.zeros((HWPAD, COLS), dtype=np.float32)
    Gp[:HW] = Ghw
    G = np.ascontiguousarray(
        Gp.reshape(NCHUNK, KC, COLS).transpose(1, 0, 2)).astype(np_dt)

    fmhw = fm_b.reshape(C, HW).T                              # [(h,w), c]
    Fp = np.zeros((HWPAD, C), dtype=np.float32)
    Fp[:HW] = fmhw
    FM = np.ascontiguousarray(
        Fp.reshape(NCHUNK, KC, C).transpose(1, 0, 2)).astype(np_dt)
    return FM, G


def _unpack_core_out(OUT):
    """OUT [128, 2, 2, 392] -> [16, 256, 7, 7]."""
    a = OUT.reshape(128, 2, 2, COLH).transpose(1, 0, 2, 3).reshape(C, COLS)
    a = a.reshape(C, GROUPS_PER_CORE, P, P).transpose(1, 0, 2, 3)
    return np.ascontiguousarray(a)


# ---------------------------------------------------------------- program

_PROGRAMS = {}


def _build_program(dt_name):
    import concourse.bacc as bacc
    import concourse.tile as tile
    import concourse.mybir as mybir

    f32 = mybir.dt.float32
    dts = {"float32": mybir.dt.float32, "float32r": mybir.dt.float32r,
           "bfloat16": mybir.dt.bfloat16}
    io_dt = dts[dt_name]

    nc = bacc.Bacc("TRN2", target_bir_lowering=False, debug=False,
                   enable_asserts=False)
    fm_d = nc.dram_tensor("fm", [KC, NCHUNK, C], io_dt, kind="ExternalInput").ap()
    g_d = nc.dram_tensor("g", [KC, NCHUNK, COLS], io_dt, kind="ExternalInput").ap()
    out_d = nc.dram_tensor("out", [128, 2, 2, COLH], f32,
                           kind="ExternalOutput").ap()

    with tile.TileContext(nc) as tc:
        with tc.tile_pool(name="fmp", bufs=NSUPER) as fmp, \
             tc.tile_pool(name="gp", bufs=NSUPER) as gpool, \
             tc.tile_pool(name="outp", bufs=1) as opool, \
             tc.tile_pool(name="psp", bufs=1, space="PSUM") as psp:

            # stream inputs in large super-chunks: fm_i then G_i so the
            # first matmul only waits on ~1/5 of the bytes
            fmt = []
            gt = []
            for i in range(NSUPER):
                Fs = fmp.tile([KC, SUPER, C], io_dt, tag="fs", name=f"fs{i}")
                nc.sync.dma_start(Fs[:], fm_d[:, i * SUPER:(i + 1) * SUPER, :])
                Gs = gpool.tile([KC, SUPER, COLS], io_dt, tag="gs", name=f"gs{i}")
                nc.sync.dma_start(Gs[:], g_d[:, i * SUPER:(i + 1) * SUPER, :])
                fmt.append(Fs)
                gt.append(Gs)

            ps = [psp.tile([128, COLH], f32, name=f"ps{i}") for i in range(4)]
            for chunk in range(NCHUNK):
                sup, j = divmod(chunk, SUPER)
                for ch in range(2):
                    lhsT = fmt[sup][:, j, ch * 128:(ch + 1) * 128]
                    for colh in range(2):
                        nc.tensor.matmul(
                            ps[ch * 2 + colh][:],
                            lhsT,
                            gt[sup][:, j, colh * COLH:(colh + 1) * COLH],
                            start=(chunk == 0), stop=(chunk == NCHUNK - 1))

            OUT = opool.tile([128, 2, 2, COLH], f32)
            for ch in range(2):
                for colh in range(2):
                    i = ch * 2 + colh
                    if i % 2 == 0:
                        nc.vector.tensor_copy(out=OUT[:, ch, colh, :],
                                              in_=ps[i][:])
                    else:
                        nc.scalar.copy(out=OUT[:, ch, colh, :], in_=ps[i][:])
            nc.sync.dma_start(out_d, OUT[:])

    nc.compile()
    return nc


LAST_RESULT = None


def _ensure_axon_hooks_shim():
    """concourse's axon trace path imports antenv.axon_hooks, which this
    image's antenv package lacks; provide a minimal registry so a stray
    BASS_TRACE=1 in the environment cannot crash the kernel."""
    try:
        import antenv  # noqa: F401
        import antenv.axon_hooks  # noqa: F401
        return
    except ImportError:
        pass
    try:
        import sys
        import types
        import antenv
        mod = types.ModuleType("antenv.axon_hooks")
        mod._hook = None
        mod.get_axon_ntff_profile_hook = lambda: mod._hook

        def _set(h):
            mod._hook = h

        mod.set_axon_ntff_profile_hook = _set
        sys.modules["antenv.axon_hooks"] = mod
        antenv.axon_hooks = mod
    except Exception:
        pass


def kernel(feature_map, boxes, gt_boxes):
    global LAST_RESULT
    _ensure_axon_hooks_shim()
    feature_map = np.asarray(feature_map, dtype=np.float32)
    boxes = np.asarray(boxes, dtype=np.float32)
    gt_boxes = np.asarray(gt_boxes, dtype=np.float32)

    from concourse.bass_utils import run_bass_kernel_spmd

    dt_name = os.environ.get("ROI_DTYPE", "bfloat16")
    if dt_name == "bfloat16":
        import ml_dtypes
        np_dt = ml_dtypes.bfloat16
    else:
        np_dt = np.float32

    if dt_name not in _PROGRAMS:
        _PROGRAMS[dt_name] = _build_program(dt_name)
    nc = _PROGRAMS[dt_name]

    in_maps = []
    for k in range(NCORES):
        b = k // 4
        g0 = (k % 4) * GROUPS_PER_CORE
        FM, G = _prep_core(feature_map[b], boxes[b], gt_boxes[b], g0, np_dt)
        in_maps.append({"fm": FM, "g": G})

    trace = bool(int(os.environ.get("ROI_TRACE", "0")))
    res = run_bass_kernel_spmd(nc, in_maps, list(range(NCORES)), trace=trace)
    LAST_RESULT = res

    out = np.zeros((B, N, C, P, P), dtype=np.float32)
    for k in range(NCORES):
        b = k // 4
        g0 = (k % 4) * GROUPS_PER_CORE
        out[b, g0:g0 + GROUPS_PER_CORE] = _unpack_core_out(res.results[k]["out"])
    return out


# revision 7
# speedup vs baseline: 10.5723x; 1.2146x over previous
"""ContextualRoIAlign Trainium2 kernel — fused group-kernel formulation.

Problem (hardcoded): B=2, C=256, H=W=56, N=64 boxes, M=8 gt boxes, P=7.
out[b,n,c,p,q] = roi_align(fm[b], box_n)[c,p,q]
                 + mean_m roi_align(fm[b], union(box_n, gt_m))[c,p,q]

roi_align separates per axis into interpolation matrices Ay, Ax
([7,dim], host-precomputed exactly like the reference), so each roi is
out_r = Ay_r @ fm @ Ax_r^T.  The whole 9-roi group sum (box + mean of
its 8 ctx unions, 1/M folded into Ax) collapses into ONE dense spatial
kernel per group:

    G_g[(h,w),(p,q)] = sum_j Ay_j[p,h] * Ax_j[q,w]          (host, ~44 MFLOP/core)
    out_g[c,(p,q)]   = sum_hw fm[c,(h,w)] * G_g[(h,w),(p,q)] (device)

The device then does a single [256 x 3136] @ [3136 x 784] matmul per
core at full 128x128 PE utilization: hw is chunked into 25 K-tiles of
128 accumulated in PSUM; fm chunk is the stationary operand (shared by
all 16 groups); G streams as the moving operand (784 = 16 groups x 49
output pixels, split in two 392-column halves to fit a PSUM bank).
~100 matmuls total instead of the ~2800 tiny per-roi matmuls of the
two-stage formulation.

Sharding: 8 cores; core k handles image k//4, box groups [16*(k%4), +16).

DMA: G ([3136, 784]) is the dominant stream; it is shipped in 5 large
super-chunks (5 K-tiles each) to stay near peak HBM bandwidth while
overlapping with PE compute. Default io dtype bf16 (G/fm magnitudes are
O(1); psum accumulates fp32); ROI_DTYPE=float32r keeps full precision
at 2x the DMA cost.
"""
import os
import numpy as np

P = 7
B, C, H, W, N, M = 2, 256, 56, 56, 64, 8
NCORES = 8
GROUPS_PER_CORE = 16
ROIS_PER_GROUP = 9          # 1 box + 8 ctx unions
R_CORE = GROUPS_PER_CORE * ROIS_PER_GROUP   # 144
HW = H * W                  # 3136
KC = 128                    # contraction tile (partition dim)
NCHUNK = 25                 # ceil(3136/128)
HWPAD = NCHUNK * KC         # 3200
# K-tiles per DMA super-chunk: small first chunk so the PE starts early,
# large steady-state chunks for DMA efficiency
SUPERS = (1, 4, 5, 5, 5, 5)
NSUPER = len(SUPERS)
PQ = P * P                  # 49
COLS = GROUPS_PER_CORE * PQ             # 784 moving columns
COLH = COLS // 2                        # 392 (<=512 psum bank)


# ---------------------------------------------------------------- host prep

def _axis_weights(start, length, dim):
    """Exact numpy port of the reference's _axis_weights (float32)."""
    start = start.astype(np.float32)
    length = length.astype(np.float32)
    R = start.shape[0]
    S = int(np.ceil(dim / P))
    bin_sz = length / np.float32(P)
    grid = np.ceil(length / np.float32(P)).astype(np.int32)
    g = grid.astype(np.float32)[:, None, None]
    s = np.arange(S, dtype=np.float32)
    ph = np.arange(P, dtype=np.float32)
    coord = (start[:, None, None] + ph[None, :, None] * bin_sz[:, None, None]
             + (s[None, None, :] + np.float32(0.5)) * bin_sz[:, None, None] / g)
    valid = (coord >= -1.0) & (coord <= dim)
    c = np.maximum(coord, np.float32(0.0))
    low = np.floor(c).astype(np.int32)
    hi_clamp = low >= dim - 1
    low = np.where(hi_clamp, dim - 1, low)
    high = np.where(hi_clamp, dim - 1, low + 1)
    cv = np.where(hi_clamp, low.astype(np.float32), c)
    l = cv - low.astype(np.float32)
    smask = (s[None, None, :] < g) & valid
    w = smask.astype(np.float32) / g
    w_low = ((np.float32(1.0) - l) * w).astype(np.float32)
    w_high = (l * w).astype(np.float32)
    A = np.zeros((R, P, dim), dtype=np.float32)
    r_idx = np.broadcast_to(np.arange(R)[:, None, None], low.shape)
    p_idx = np.broadcast_to(np.arange(P)[None, :, None], low.shape)
    np.add.at(A, (r_idx, p_idx, low), w_low)
    np.add.at(A, (r_idx, p_idx, high), w_high)
    return A


def _prep_core(fm_b, boxes_b, gt_b, g0, np_dt):
    """Per-core inputs: FM [128, 25, 256], G [128, 25, 784]."""
    b = boxes_b.astype(np.float32)
    g = gt_b.astype(np.float32)
    x1 = np.minimum(b[:, None, 0], g[None, :, 0])
    y1 = np.minimum(b[:, None, 1], g[None, :, 1])
    x2 = np.maximum(b[:, None, 2], g[None, :, 2])
    y2 = np.maximum(b[:, None, 3], g[None, :, 3])
    ctx = np.stack([x1, y1, x2, y2], axis=-1)                 # [N,M,4]
    rois = np.concatenate([b[:, None, :], ctx], axis=1)       # [N,9,4]
    wts = np.full((N, ROIS_PER_GROUP), np.float32(1.0 / M), dtype=np.float32)
    wts[:, 0] = np.float32(1.0)

    rois = rois[g0:g0 + GROUPS_PER_CORE].reshape(R_CORE, 4)
    wts = wts[g0:g0 + GROUPS_PER_CORE].reshape(R_CORE)
    x1, y1, x2, y2 = rois[:, 0], rois[:, 1], rois[:, 2], rois[:, 3]
    roi_w = np.maximum(x2 - x1, np.float32(1.0))
    roi_h = np.maximum(y2 - y1, np.float32(1.0))
    Ay = _axis_weights(y1, roi_h, H)                          # [144,7,56]
    Ax = _axis_weights(x1, roi_w, W) * wts[:, None, None]     # [144,7,56]

    # G_g[p,h,q,w] = sum_j Ay[j,p,h] Ax[j,q,w]  (rank-9 per group)
    Ayg = Ay.reshape(GROUPS_PER_CORE, ROIS_PER_GROUP, P * H)
    Axg = Ax.reshape(GROUPS_PER_CORE, ROIS_PER_GROUP, P * W)
    G2 = np.matmul(Ayg.transpose(0, 2, 1), Axg)               # [16, 392, 392]
    G5 = G2.reshape(GROUPS_PER_CORE, P, H, P, W)
    # -> [(h,w), (g,p,q)]
    Ghw = np.ascontiguousarray(G5.transpose(2, 4, 0, 1, 3)).reshape(HW, COLS)
    Gp = np.zeros((HWPAD, COLS), dtype=np.float32)
    Gp[:HW] = Ghw
    G = np.ascontiguousarray(
        Gp.reshape(NCHUNK, KC, COLS).transpose(1, 0, 2)).astype(np_dt)

    fmhw = fm_b.reshape(C, HW).T                              # [(h,w), c]
    Fp = np.zeros((HWPAD, C), dtype=np.float32)
    Fp[:HW] = fmhw
    FM = np.ascontiguousarray(
        Fp.reshape(NCHUNK, KC, C).transpose(1, 0, 2)).astype(np_dt)
    return FM, G


def _unpack_core_out(OUT):
    """OUT [128, 2, 2, 392] -> [16, 256, 7, 7]."""
    a = np.asarray(OUT, dtype=np.float32)
    a = a.reshape(128, 2, 2, COLH).transpose(1, 0, 2, 3).reshape(C, COLS)
    a = a.reshape(C, GROUPS_PER_CORE, P, P).transpose(1, 0, 2, 3)
    return np.ascontiguousarray(a)


# ---------------------------------------------------------------- program

_PROGRAMS = {}


def _build_program(dt_name):
    import concourse.bacc as bacc
    import concourse.tile as tile
    import concourse.mybir as mybir

    f32 = mybir.dt.float32
    dts = {"float32": mybir.dt.float32, "float32r": mybir.dt.float32r,
           "bfloat16": mybir.dt.bfloat16}
    io_dt = dts[dt_name]

    nc = bacc.Bacc("TRN2", target_bir_lowering=False, debug=False,
                   enable_asserts=False)
    fm_d = nc.dram_tensor("fm", [KC, NCHUNK, C], io_dt, kind="ExternalInput").ap()
    g_d = nc.dram_tensor("g", [KC, NCHUNK, COLS], io_dt, kind="ExternalInput").ap()
    out_d = nc.dram_tensor("out", [128, 2, 2, COLH], io_dt,
                           kind="ExternalOutput").ap()

    with tile.TileContext(nc) as tc:
        with tc.tile_pool(name="fmp", bufs=1) as fmp, \
             tc.tile_pool(name="gp", bufs=1) as gpool, \
             tc.tile_pool(name="outp", bufs=1) as opool, \
             tc.tile_pool(name="psp", bufs=1, space="PSUM") as psp:

            # stream inputs in super-chunks, alternating the two HWDGE
            # rings (sync=SP, scalar=Act) so one ring's transfer hides the
            # other's ~2us completion latency; G_i and fm_i ride opposite
            # rings so they land together
            fmt = []
            gt = []
            c0 = 0
            for i, s in enumerate(SUPERS):
                qa = nc.sync if i % 2 == 0 else nc.scalar
                qb = nc.scalar if i % 2 == 0 else nc.sync
                Fs = fmp.tile([KC, s, C], io_dt, name=f"fs{i}")
                qa.dma_start(Fs[:], fm_d[:, c0:c0 + s, :])
                Gs = gpool.tile([KC, s, COLS], io_dt, name=f"gs{i}")
                qb.dma_start(Gs[:], g_d[:, c0:c0 + s, :])
                fmt.append(Fs)
                gt.append(Gs)
                c0 += s

            ps = [psp.tile([128, COLH], f32, name=f"ps{i}") for i in range(4)]
            OUTt = [opool.tile([128, COLH], io_dt, name=f"out{i}")
                    for i in range(4)]
            chunk = 0
            for sup, s in enumerate(SUPERS):
                for j in range(s):
                    for ch in range(2):
                        lhsT = fmt[sup][:, j, ch * 128:(ch + 1) * 128]
                        for colh in range(2):
                            i = ch * 2 + colh
                            nc.tensor.matmul(
                                ps[i][:],
                                lhsT,
                                gt[sup][:, j, colh * COLH:(colh + 1) * COLH],
                                start=(chunk == 0), stop=(chunk == NCHUNK - 1))
                            # drain each psum tile as soon as its
                            # accumulation closes (overlaps the remaining
                            # matmuls + hides the store DMA)
                            if chunk == NCHUNK - 1:
                                ch_, colh_ = divmod(i, 2)
                                if i % 2 == 0:
                                    nc.vector.tensor_copy(out=OUTt[i][:],
                                                          in_=ps[i][:])
                                    nc.sync.dma_start(
                                        out_d[:, ch_, colh_, :], OUTt[i][:])
                                else:
                                    nc.scalar.copy(out=OUTt[i][:], in_=ps[i][:])
                                    nc.scalar.dma_start(
                                        out_d[:, ch_, colh_, :], OUTt[i][:])
                    chunk += 1

    nc.compile()
    return nc


LAST_RESULT = None


def _ensure_axon_hooks_shim():
    """concourse's axon trace path imports antenv.axon_hooks, which this
    image's antenv package lacks; provide a minimal registry so a stray
    BASS_TRACE=1 in the environment cannot crash the kernel."""
    try:
        import antenv  # noqa: F401
        import antenv.axon_hooks  # noqa: F401
        return
    except ImportError:
        pass
    try:
        import sys
        import types
        import antenv
        mod = types.ModuleType("antenv.axon_hooks")
        mod._hook = None
        mod.get_axon_ntff_profile_hook = lambda: mod._hook

        def _set(h):
            mod._hook = h

        mod.set_axon_ntff_profile_hook = _set
        sys.modules["antenv.axon_hooks"] = mod
        antenv.axon_hooks = mod
    except Exception:
        pass


def kernel(feature_map, boxes, gt_boxes):
    global LAST_RESULT
    _ensure_axon_hooks_shim()
    feature_map = np.asarray(feature_map, dtype=np.float32)
    boxes = np.asarray(boxes, dtype=np.float32)
    gt_boxes = np.asarray(gt_boxes, dtype=np.float32)

    from concourse.bass_utils import run_bass_kernel_spmd

    dt_name = os.environ.get("ROI_DTYPE", "bfloat16")
    if dt_name == "bfloat16":
        import ml_dtypes
        np_dt = ml_dtypes.bfloat16
    else:
        np_dt = np.float32

    if dt_name not in _PROGRAMS:
        _PROGRAMS[dt_name] = _build_program(dt_name)
    nc = _PROGRAMS[dt_name]

    in_maps = []
    for k in range(NCORES):
        b = k // 4
        g0 = (k % 4) * GROUPS_PER_CORE
        FM, G = _prep_core(feature_map[b], boxes[b], gt_boxes[b], g0, np_dt)
        in_maps.append({"fm": FM, "g": G})

    trace = bool(int(os.environ.get("ROI_TRACE", "0")))
    res = run_bass_kernel_spmd(nc, in_maps, list(range(NCORES)), trace=trace)
    LAST_RESULT = res

    out = np.zeros((B, N, C, P, P), dtype=np.float32)
    for k in range(NCORES):
        b = k // 4
        g0 = (k % 4) * GROUPS_PER_CORE
        out[b, g0:g0 + GROUPS_PER_CORE] = _unpack_core_out(res.results[k]["out"])
    return out
